# revision 52
# baseline (speedup 1.0000x reference)
"""Trainium2 Bass kernel for nn_MiniDecoderBlock (B=2, T=2048, D=1024, H=16, DI=2048).

Strategy: 8-way tensor-parallel attention (2 heads/core, both batches),
one chunked ReduceScatter of the o_proj partial sums distributing tokens,
then token-sharded FFN (512 tokens/core, full d_inner).

kernel(**inputs) takes the FULL unsharded inputs and returns the FULL
output; sharding/compile/run happen inside.
"""

"""MiniDecoderBlock Trainium kernel: TP-8 attention + RS + token-sharded FFN.

Layout conventions (device side, per core):
  - Activations feature-major: xT [D, tokens] so matmul contraction (partition
    dim) is the feature dim.
  - Scores computed transposed: scoresT [k_tokens(P), q_tokens(free)] so the
    PV matmul uses stationary V and lands yT feature-major for o_proj.
  - V stored token-major with an appended ones column (sumexp for free).
  - rmsnorm applied via a PE ones-broadcast of the rms row onto all partitions,
    multiplied into q/k/v at the mandatory PSUM->SBUF copy.
  - ReduceScatter distributes attention partial sums by token blocks; core r
    owns global 128-token blocks {8c + r}.
"""

import numpy as np

import concourse.bass as bass
import concourse.mybir as mybir
import concourse.tile as tile
from concourse import bacc
from concourse.masks import make_identity
from concourse.tile import TileContext

F32 = mybir.dt.float32
F32R = mybir.dt.float32r
F8 = mybir.dt.float8e4
BF16 = mybir.dt.bfloat16
DR = mybir.MatmulPerfMode.DoubleRow

N_CORES = 8
B, T, D = 2, 2048, 1024
H, HD = 16, 64
DI = 2048
HPC = H // N_CORES          # heads per core = 2
NTOK = B * T                # 4096
NCHUNK = NTOK // 512        # 8 x 512-token chunks
NBLK = NTOK // 128          # 32 x 128-token blocks
EPS = 1e-6
NEG = -1e30
SQKV = 64.0
NEG_LN_SQKV = -np.log(SQKV)


def r32(ap):
    return ap.bitcast(F32R)



def _pin_act_tables():
    import concourse.bacc as bacc_mod
    import concourse.hw_specs as hw_specs_mod
    import concourse.mybir as _mb
    orig = hw_specs_mod.get_activation_tables
    if getattr(bacc_mod.get_activation_tables, "_pinned", False):
        return
    AFT = _mb.ActivationFunctionType
    def patched(arch):
        t = orig(arch)
        out = {}
        for k, v in t.items():
            if k == "natural_log_exp_and_others":
                out[k] = set(v)
            else:
                out[k] = {f for f in v if f not in (AFT.Ln, AFT.Exp)}
        return out
    patched._pinned = True
    bacc_mod.get_activation_tables = patched

def build_nc(ffn_w_dtype=BF16, reps=1, no_collective=False):
    _pin_act_tables()
    nc = bacc.Bacc("TRN2", target_bir_lowering=False, debug=False,
                   num_devices=1 if no_collective else N_CORES)

    xT = nc.dram_tensor("xT", [NCHUNK, 128, 8 * 512], F8, kind="ExternalInput")
    x_own = nc.dram_tensor("x_own", [512, D], BF16, kind="ExternalInput")
    qkvT = nc.dram_tensor("qkvT", [D, 3 * HPC * HD], F8, kind="ExternalInput")
    o_wT = nc.dram_tensor("o_wT", [D, D], F8, kind="ExternalInput")
    gT = nc.dram_tensor("gT", [D, DI], F8, kind="ExternalInput")
    uT = nc.dram_tensor("uT", [D, DI], F8, kind="ExternalInput")
    dT = nc.dram_tensor("dT", [DI, D], F8, kind="ExternalInput")
    out = nc.dram_tensor("out", [512, D], F32, kind="ExternalOutput")

    with TileContext(nc) as tc:
        emit(nc, tc, xT, x_own, qkvT, o_wT, gT, uT, dT, out, reps=reps,
             no_collective=no_collective)
    nc.compile()
    return nc


def emit(nc, tc, xT, x_own, qkvT, o_wT, gT, uT, dT, out, reps=1, no_collective=False):
    EXP = mybir.ActivationFunctionType.Exp
    LN = mybir.ActivationFunctionType.Ln
    SQUARE = mybir.ActivationFunctionType.Square
    SILU = mybir.ActivationFunctionType.Silu
    MUL = mybir.AluOpType.mult
    ADD = mybir.AluOpType.add

    from contextlib import ExitStack
    ctx = ExitStack()
    consts = ctx.enter_context(tc.tile_pool(name="consts", bufs=1))
    dram = ctx.enter_context(tc.tile_pool(name="dram", bufs=1, space="DRAM"))
    psum = ctx.enter_context(tc.tile_pool(name="psum", bufs=2, space="PSUM"))
    sb = ctx.enter_context(tc.tile_pool(name="sb", bufs=2))

    # ---- constants ----
    ident_bf = consts.tile([128, 128], BF16, tag="ident_bf")
    make_identity(nc, ident_bf[:, :])
    eps_col = consts.tile([128, 1], F32, tag="eps_col")
    nc.vector.memset(eps_col[:, :], EPS)
    nl64_col = consts.tile([128, 1], F32, tag="nl64_col")
    nc.vector.memset(nl64_col[:, :], NEG_LN_SQKV)
    # PE-side causal masking: scores_diag = (-BIG*I).T @ ut01 + kT.T @ q
    # neg_ident = -BIG on the diagonal; ut01 = 1.0 strictly below diagonal (k>q)
    maddT = consts.tile([128, 128], BF16, tag="maddT")
    nc.gpsimd.memset(maddT[:, :], 0.0)
    nc.gpsimd.affine_select(
        out=maddT[:, :], in_=maddT[:, :],
        compare_op=mybir.AluOpType.is_ge, fill=NEG,
        base=0, pattern=[[-1, 128]], channel_multiplier=1,
    )

    # ---- persistent SBUF ----
    # Single DMAs with (p, blk, col)-ordered DRAM access patterns so each
    # load is one HWDGE instruction (~667ns queue cost each) instead of 8+.
    xo_sb = consts.tile([128, 4, D], BF16, tag="xo_sb")
    qkvT_sb = consts.tile([128, 8, 384], F8, tag="qkvT_sb")
    nc.sync.dma_start(out=qkvT_sb[:, :, :],
                      in_=qkvT[:, :].rearrange("(kk p) c -> p kk c", p=128))
    # full o_w resident (same on every core), DoubleRow pair layout over the
    # y-dim: [:, pp, j, :] = rows (2pp+j)*128..(2pp+j+1)*128 of o_w.T
    # (loaded at i==0 so it stays off the startup critical path)
    o_wT_sb = consts.tile([128, 4, 2, D], F8, tag="o_wT_sb")

    kT_all = consts.tile([128, NTOK], BF16, tag="kT_all")
    # fp8 V, token-major, 80-wide slots (pair stride %16==0 for dual-fp8 LW): [:, h, gb, 0:64] = v for block gb,
    # col 64 = 1.0 (sumexp row). Adjacent blocks pair-slice for DoubleRow PV.
    v2 = consts.tile([128, HPC, NBLK, 80], F8, tag="v2")
    nc.vector.memset(v2[:, :, :, 64:65], 1.0)

    # FFN weights resident, fp8, DoubleRow pair layout: [:, p, j, :] holds
    # feature rows (2p+j)*128..(2p+j+1)*128 so lhsT slices [128,2,128] pair
    # two contraction blocks per matmul.
    g_sb = consts.tile([128, 4, 2, DI], gT.dtype, tag="g_sb")
    u_sb = consts.tile([128, 4, 2, DI], uT.dtype, tag="u_sb")
    d_sb = consts.tile([128, 8, 2, D], dT.dtype, tag="d_sb")

    # ---- DRAM bounce ----
    # y2 AllToAll: per chunk-pair c, core r receives every core's 2 heads of
    # y for ITS token block 8c+r. a2a[c, s] = [128 y-rows, 128 tokens].
    a2a_in = dram.tile([4, N_CORES, 128, 128], F8, tag="a2a_in")
    a2a_out = dram.tile([4, N_CORES, 128, 128], F8, tag="a2a_out")
    rms_own = dram.tile([4, 128], F32, tag="rms_own")
    # rms_all[r, c, :] = rms of global token block 8c+r (AllGather core-major)
    rms_all = dram.tile([8, 4, 128], F32, tag="rms_all")


    for _rep in range(reps):
        # ================= main loop over 512-token chunks =================
        def xt_load(i):
            # host pre-arranged to the exact SBUF layout: 4KB descriptors
            xt = sb.tile([128, 8, 512], F8, tag="xt", name=f"xt_{i}")
            nc.sync.dma_start(out=xt[:, :, :], in_=xT[i, :, :])
            return xt

        def rms_part(i):
            rms_row = sb.tile([1, 512], F32, tag="rms_row", name=f"rmsr_{i}")
            nc.sync.dma_start(
                out=rms_row[0:1, :],
                in_=rms_all[(4 * i) % 8:(4 * i) % 8 + 4, i // 2, :])
            rms_b = sb.tile([128, 512], F32, tag="rms_b", name=f"rmsb_{i}")
            nc.gpsimd.partition_broadcast(rms_b[:, :], rms_row[0:1, :])
            return rms_b

        def stats(i):
            """Load xT chunk i + rms broadcast tile (emitted ~1.5 chunks
            ahead); rms values come from the AllGathered per-core stats."""
            return xt_load(i), rms_part(i)

        def qkv_steps(i, st):
            """Projection for chunk i as filler closures sprinkled into the
            previous chunk's attention g-loop (PE fills exp-wait gaps)."""
            csl = slice(i * 512, (i + 1) * 512)
            xt, rms_b = st
            state = {}
            steps = []

            def mk_proj(w, off, k2):
                def f():
                    if k2 == 0:
                        state[w] = psum.tile([128, 512], F32, tag="proj",
                                             bufs=2, name=f"pj_{w}_{i}")
                    nc.tensor.matmul(
                        state[w][:, :],
                        qkvT_sb[:, 2 * k2:2 * k2 + 2, off:off + 128],
                        xt[:, 2 * k2:2 * k2 + 2, :],
                        start=(k2 == 0), stop=(k2 == 3),
                        perf_mode=DR)
                return f

            def mk_epi(w):
                # per-projection epilogue so the proj psum ring can be 2 deep
                def f():
                    if w == "q":
                        q_sb = sb.tile([128, 512], BF16, tag="q_sb",
                                       name=f"q_{i}")
                        state["q_sb"] = q_sb
                        nc.vector.tensor_tensor(out=q_sb[:, :],
                                                in0=state["q"][:, :],
                                                in1=rms_b[:, :], op=MUL)
                    elif w == "k":
                        nc.vector.tensor_tensor(out=kT_all[:, csl],
                                                in0=state["k"][:, :],
                                                in1=rms_b[:, :], op=MUL)
                    else:
                        v_sb = sb.tile([128, 512], BF16, tag="v_sb",
                                       name=f"v_{i}")
                        state["v_sb"] = v_sb
                        nc.vector.tensor_tensor(out=v_sb[:, :],
                                                in0=state["v"][:, :],
                                                in1=rms_b[:, :], op=MUL)
                return f

            for w, off in (("q", 0), ("k", 128), ("v", 256)):
                for k2 in range(4):
                    steps.append(mk_proj(w, off, k2))
                steps.append(mk_epi(w))

            def mk_vt(h, j):
                def f():
                    gb = i * 4 + j
                    v_sb = state["v_sb"]
                    vt = psum.tile([128, 64], BF16, tag="proj", bufs=2)
                    nc.tensor.transpose(vt[:, :],
                                        v_sb[h * 64:(h + 1) * 64,
                                             j * 128:(j + 1) * 128],
                                        ident_bf[h * 64:(h + 1) * 64,
                                                  h * 64:(h + 1) * 64])
                    nc.vector.tensor_copy(v2[:, h, gb, 0:64], vt[:, :])
                return f

            for h in range(HPC):
                for j in range(4):
                    steps.append(mk_vt(h, j))
            return steps, state

        def attn_both(i, q_sb, y2_sb, fillers=()):
            fillers = list(fillers)
            b, li = divmod(i, 4)
            npair = 2 * li
            nstep = npair + 4
            per_g = max(1, -(-len(fillers) // nstep))
            yT = [psum.tile([128, 512], F32, tag="yT", bufs=2, name=f"yT_{i}_{h}")
                  for h in range(2)]

            def pop_some():
                for _ in range(per_g):
                    if fillers:
                        fillers.pop(0)()

            # ---- full k-block pairs: one exp per (head, pair), fp8 DR PV --
            for pi in range(npair):
                gb0 = b * 16 + 2 * pi
                scs = []
                for h in range(2):
                    scp = psum.tile([128, 2, 512], F32, tag="sc", bufs=2,
                                    name=f"scp{h}")
                    for jj in range(2):
                        nc.tensor.matmul(
                            scp[:, jj, :],
                            kT_all[h * 64:(h + 1) * 64,
                                   (gb0 + jj) * 128:(gb0 + jj + 1) * 128],
                            q_sb[h * 64:(h + 1) * 64, :],
                            start=True, stop=True)
                    scs.append(scp)
                for h in range(2):
                    pT = sb.tile([128, 2, 512], F8, tag="pT", bufs=3,
                                 name=f"pT{h}")
                    nc.scalar.activation(pT[:, :, :], scs[h][:, :, :], EXP)
                    nc.tensor.matmul(
                        yT[h][0:65, :],
                        v2[:, h, gb0:gb0 + 2, 0:65],
                        pT[:, :, :],
                        start=(pi == 0), stop=False, perf_mode=DR)
                pop_some()

            # ---- diagonal blocks: both heads share one psum pair-tile ----
            for d in range(4):
                g = li * 4 + d
                gb = b * 16 + g
                q_off = d * 128
                w = 512 - q_off
                sc = psum.tile([128, 2, 512], F32, tag="sc", bufs=2,
                               name="scd")
                for h in range(2):
                    nc.tensor.matmul(
                        sc[:, h, 0:w],
                        kT_all[h * 64:(h + 1) * 64, gb * 128:(gb + 1) * 128],
                        q_sb[h * 64:(h + 1) * 64, q_off:512],
                        start=True, stop=False)
                    nc.tensor.matmul(
                        sc[:, h, 0:128], maddT[:, :], ident_bf[:, :],
                        start=False, stop=True)
                pT = sb.tile([128, 2, 512], F8, tag="pT", bufs=3, name="pTd")
                nc.scalar.activation(pT[:, :, 0:w], sc[:, :, 0:w], EXP)
                for h in range(2):
                    nc.tensor.matmul(
                        yT[h][0:65, q_off:512],
                        v2[:, h, gb, 0:65],
                        pT[:, h, 0:w],
                        start=(li == 0 and d == 0), stop=(d == 3))
                pop_some()
            while fillers:
                fillers.pop(0)()
            for h in range(2):
                rec = sb.tile([1, 512], F32, tag="rec")
                nc.vector.reciprocal(rec[:, :], yT[h][64:65, :])
                rb = sb.tile([64, 512], F32, tag="rb", bufs=1)
                nc.gpsimd.partition_broadcast(rb[:, :], rec[0:1, :])
                # y2 = 16*y/sumexp (x16 for fp8 range; undone by the /1024)
                nc.vector.scalar_tensor_tensor(
                    out=y2_sb[:, h, :], in0=yT[h][0:64, :], scalar=16.0,
                    in1=rb[:, :], op0=MUL, op1=MUL)

        def y2_out(i, y2_sb):
            # chunk i holds blocks 4i..4i+3 -> destination cores 4(i%2)+jb of
            # the pair i//2; one DMA scatters all four.
            c, half = divmod(i, 2)
            for jb in range(4):
                nc.sync.dma_start(
                    out=a2a_in[c, 4 * half + jb].rearrange(
                        "(h p) t -> p h t", p=64),
                    in_=y2_sb[:, :, jb * 128:(jb + 1) * 128])

        def emit_a2a(c):
            if no_collective:
                nc.sync.dma_start(out=a2a_out[c], in_=a2a_in[c])
            else:
                nc.gpsimd.collective_compute(
                    "AllToAll", mybir.AluOpType.bypass,
                    ins=[a2a_in[c]],
                    outs=[a2a_out[c]],
                    replica_groups=[list(range(N_CORES))],
                )

        # ================= FFN on own 512 tokens =================
        def ffn_load(ha):
            """Fetch the AllToAll'd full-head y for own blocks ha*2, ha*2+1
            (a2a_out[c2] holds [8 src cores, 128 y-rows, 128 own tokens])."""
            loads = []
            for jj in range(2):
                c2 = ha * 2 + jj
                y_sb = sb.tile([128, 4, 2, 128], F8, tag="y_sb", bufs=2,
                               name=f"ysb{c2}")
                nc.sync.dma_start(
                    out=y_sb[:, :, :, :],
                    in_=a2a_out[c2].rearrange("(pp j) p t -> p pp j t", j=2))
                loads.append(y_sb)
            return loads

        def ffn_prep(ha, loads):
            x2t, xn2T, xn2s = [], [], []
            for jj in range(2):
                c2 = ha * 2 + jj
                y_sb = loads[jj]
                op2 = psum.tile([128, 2, 512], F32, tag="sc", bufs=2,
                                name=f"op2_{c2}")
                for n in range(2):
                    for pp in range(4):
                        nc.tensor.matmul(
                            op2[:, n, :],
                            y_sb[:, pp, :, :],
                            o_wT_sb[:, pp, :, n * 512:(n + 1) * 512],
                            start=(pp == 0), stop=(pp == 3),
                            perf_mode=DR)
                x2 = sb.tile([128, D], BF16, tag=f"x2_{jj}", bufs=2,
                             name=f"x2_{c2}")
                nc.vector.scalar_tensor_tensor(
                    out=x2[:, :], in0=op2[:, :, :], scalar=1.0 / 1024.0,
                    in1=xo_sb[:, c2, :], op0=MUL, op1=ADD)
                x2t.append(x2)
                scr = sb.tile([128, D], BF16, tag="scr", bufs=1, name=f"scr{c2}")
                ss2 = sb.tile([128, 1], F32, tag="ss2", name=f"ss2_{c2}")
                nc.scalar.activation(scr[:, :], x2[:, :], SQUARE,
                                     accum_out=ss2[:, :])
                t2 = sb.tile([128, 1], F32, tag="t2", name=f"t2_{c2}")
                nc.scalar.activation(t2[:, :], ss2[:, :], LN,
                                     bias=eps_col[:, :], scale=1.0 / D)
                r2 = sb.tile([128, 1], F32, tag="r2", name=f"r2_{c2}")
                nc.scalar.activation(r2[:, :], t2[:, :], EXP, scale=-0.5)
                xn2 = sb.tile([128, D], BF16, tag=f"xn2_{jj}", bufs=2,
                              name=f"xn2_{c2}")
                nc.vector.tensor_scalar_mul(xn2[:, :], x2[:, :], r2[:, :])
                xn2s.append(xn2)
            for p in range(4):
                xt2 = sb.tile([128, 2, 256], gT.dtype, tag=f"xn2T{p}",
                              bufs=2, name=f"xn2T{p}_{ha}")
                xn2T.append(xt2)

            def mk_tp(jj, kk):
                def f():
                    xp = psum.tile([128, 128], BF16, tag="proj", bufs=2)
                    nc.tensor.transpose(xp[:, :],
                                        xn2s[jj][:, kk * 128:(kk + 1) * 128],
                                        ident_bf[:, :])
                    nc.vector.tensor_copy(
                        xn2T[kk // 2][:, kk % 2, jj * 128:(jj + 1) * 128],
                        xp[:, :])
                return f

            tps = [mk_tp(jj, kk) for jj in range(2) for kk in range(8)]
            return x2t, xn2T, tps

        def ffn_gu_steps(ha, xn2T, gu_tag="sc"):
            """Gate/up m-loop as filler closures; gp+up share one psum bank
            (pipeline depth 2 within a 2-buf ring)."""
            h_all = sb.tile([128, 16, 256], F8, tag=f"h_all{ha}", bufs=1,
                            name=f"h_all_{ha}")

            def mk(m):
                def f():
                    gu = psum.tile([128, 2, 256], F32, tag=gu_tag, bufs=2,
                                   name=f"gu{ha}_{m}")
                    for p in range(4):
                        nc.tensor.matmul(gu[:, 0, :],
                                         g_sb[:, p, :, m * 128:(m + 1) * 128],
                                         xn2T[p][:, :, :],
                                         start=(p == 0), stop=(p == 3),
                                         perf_mode=DR)
                    for p in range(4):
                        nc.tensor.matmul(gu[:, 1, :],
                                         u_sb[:, p, :, m * 128:(m + 1) * 128],
                                         xn2T[p][:, :, :],
                                         start=(p == 0), stop=(p == 3),
                                         perf_mode=DR)
                    sg = sb.tile([128, 256], BF16, tag="sg")
                    nc.scalar.activation(sg[:, :], gu[:, 0, :], SILU,
                                         scale=1.0 / 64.0)
                    # h scaled x16 for fp8 (corrected with the down x64)
                    nc.vector.scalar_tensor_tensor(
                        out=h_all[:, m, :], in0=sg[:, :], scalar=0.5,
                        in1=gu[:, 1, :], op0=MUL, op1=MUL)
                return f

            return h_all, [mk(m) for m in range(16)]

        def ffn_down(ha, x2t, h_all):
            dp = [psum.tile([128, 512], F32,
                            tag="proj" if nn == 0 else "sc", bufs=2,
                            name=f"dp{jj}_{nn}")
                  for nn in range(2) for jj in range(2)]
            for m2 in range(8):
                for n in range(2):
                    for jj in range(2):
                        nc.tensor.matmul(dp[n * 2 + jj][:, :],
                                         h_all[:, 2 * m2:2 * m2 + 2,
                                               jj * 128:(jj + 1) * 128],
                                         d_sb[:, m2, :, n * 512:(n + 1) * 512],
                                         start=(m2 == 0), stop=(m2 == 7),
                                         perf_mode=DR)
            for n in range(2):
                for jj in range(2):
                    c2 = ha * 2 + jj
                    osb = sb.tile([128, 512], F32, tag="fout")
                    nc.vector.scalar_tensor_tensor(
                        out=osb[:, :], in0=dp[n * 2 + jj][:, :],
                        scalar=1.0 / 1024.0,
                        in1=x2t[jj][:, n * 512:(n + 1) * 512],
                        op0=MUL, op1=ADD)
                    nc.sync.dma_start(out=out[c2 * 128:(c2 + 1) * 128,
                                              n * 512:(n + 1) * 512],
                                      in_=osb[:, :])


        # startup SP order: qkvT (done above), xt0, xo, xt1 -- the xt0
        # stream runs while the rms chain (xo -> squares -> AllGather) flows.
        xt0 = xt_load(0)
        nc.sync.dma_start(
            out=xo_sb[:, :, :],
            in_=x_own[:, :].rearrange("(blk p) c -> p blk c", p=128))

        # ---- per-core rms of own 512 tokens, AllGathered to every core ----
        rmsq = sb.tile([128, 4], F32, tag="rmsq", bufs=1, name="rmsq")
        for blk in range(4):
            scr0 = sb.tile([128, D], BF16, tag="scr", bufs=1,
                           name=f"scr0_{blk}")
            ss0 = sb.tile([128, 1], F32, tag="ss2", name=f"ss0_{blk}")
            nc.scalar.activation(scr0[:, :], xo_sb[:, blk, :], SQUARE,
                                 accum_out=ss0[:, :])
            t0 = sb.tile([128, 1], F32, tag="t2", name=f"t0_{blk}")
            nc.scalar.activation(t0[:, :], ss0[:, :], LN,
                                 bias=eps_col[:, :], scale=1.0 / D)
            nc.scalar.activation(rmsq[:, blk:blk + 1], t0[:, :], EXP,
                                 scale=-0.5, bias=nl64_col[:, :])
            nc.gpsimd.dma_start(out=rms_own[blk:blk + 1, :],
                                in_=rmsq[:, blk:blk + 1])
        if no_collective:
            nc.gpsimd.dma_start(out=rms_all[0:1, :, :], in_=rms_own[:, :])
        else:
            nc.gpsimd.collective_compute(
                "AllGather", mybir.AluOpType.bypass,
                ins=[rms_own[:, :]], outs=[rms_all[:, :, :]],
                replica_groups=[list(range(N_CORES))],
            )

        st = (xt0, rms_part(0))
        steps0, state0 = qkv_steps(0, st)
        for f in steps0:
            f()
        q_cur = state0["q_sb"]
        st_next = stats(1)
        state_next = None
        for i in range(NCHUNK):
            y2_sb = sb.tile([64, 2, 512], F8, tag="y2_sb", name=f"y2_{i}")
            if i + 1 < NCHUNK:
                fillers, state_next = qkv_steps(i + 1, st_next)
            else:
                fillers = []
            attn_both(i, q_cur, y2_sb, fillers)
            if i == 0:
                nc.sync.dma_start(
                    out=o_wT_sb[:, :, :, :],
                    in_=o_wT[:, :].rearrange("(pp j p) c -> p pp j c",
                                             p=128, j=2))
            if i == 1:
                nc.sync.dma_start(
                    out=g_sb[:, :, :, :],
                    in_=gT[:, :].rearrange("(pp j p) c -> p pp j c",
                                           p=128, j=2))
                nc.sync.dma_start(
                    out=u_sb[:, :, :, :],
                    in_=uT[:, :].rearrange("(pp j p) c -> p pp j c",
                                           p=128, j=2))
            if i == 2:
                nc.sync.dma_start(
                    out=d_sb[:, :, :, :],
                    in_=dT[:, :].rearrange("(m j p) c -> p m j c",
                                           p=128, j=2))
            if i + 2 < NCHUNK:
                st_next = stats(i + 2)
            y2_out(i, y2_sb)
            if i + 1 < NCHUNK:
                q_cur = state_next["q_sb"]
            if i % 2 == 1:
                emit_a2a(i // 2)
            if i == 5:
                ffn0 = ffn_prep(0, ffn_load(0))

        loads1 = ffn_load(1)        # waits on the final AllToAll (DMA only)
        x2t0, xn2T0, tps0 = ffn0
        for f in tps0:
            f()
        h_all0, gu0 = ffn_gu_steps(0, xn2T0)
        for f in gu0:
            f()
        x2t1, xn2T1, tps1 = ffn_prep(1, loads1)
        ffn_down(0, x2t0, h_all0)
        for f in tps1:
            f()
        h_all1, gu1 = ffn_gu_steps(1, xn2T1)
        for f in gu1:
            f()
        ffn_down(1, x2t1, h_all1)

    ctx.close()


# ===================== host-side sharding =====================

def make_in_maps(x, ln1_w, ln2_w, qkv_w, o_w, gate_w, up_w, down_w,
                 ffn_np_dtype=None):
    import ml_dtypes
    if ffn_np_dtype is None:
        ffn_np_dtype = ml_dtypes.bfloat16
    x = np.asarray(x, np.float32)
    xf = np.ascontiguousarray(x.reshape(NTOK, D))
    # chunk-major, SBUF-layout: xT[i, p, kk*512+t] = x[512i+t, kk*128+p]
    xT = np.ascontiguousarray(
        xf.reshape(NCHUNK, 512, 8, 128).transpose(0, 3, 2, 1).reshape(
            NCHUNK, 128, 8 * 512)).astype(ml_dtypes.float8_e4m3)

    qkv_eff = np.asarray(qkv_w, np.float32) * np.asarray(ln1_w, np.float32)[None, :]
    g_eff = np.asarray(gate_w, np.float32) * np.asarray(ln2_w, np.float32)[None, :]
    u_eff = np.asarray(up_w, np.float32) * np.asarray(ln2_w, np.float32)[None, :]
    o_w = np.asarray(o_w, np.float32)
    down_w = np.asarray(down_w, np.float32)

    gT = np.ascontiguousarray(g_eff.T * 64.0).astype(ml_dtypes.float8_e4m3)
    uT = np.ascontiguousarray(u_eff.T * 32.0).astype(ml_dtypes.float8_e4m3)
    dT = np.ascontiguousarray(down_w.T * 64.0).astype(ml_dtypes.float8_e4m3)

    scale = 1.0 / np.sqrt(HD)
    # full o_w.T (y-major), identical on every core
    o_wT_full = np.ascontiguousarray(o_w.T * 64.0).astype(
        ml_dtypes.float8_e4m3)
    in_maps = []
    for r in range(N_CORES):
        hsl = slice(r * HPC * HD, (r + 1) * HPC * HD)  # rows for this core's heads
        qr = qkv_eff[hsl, :] * scale          # [128, D] pre-scaled q
        kr = qkv_eff[D + r * 128:D + (r + 1) * 128, :]
        vr = qkv_eff[2 * D + r * 128:2 * D + (r + 1) * 128, :]
        qkvT_r = np.ascontiguousarray(
            np.concatenate([qr, kr, vr], axis=0).T * SQKV).astype(
                ml_dtypes.float8_e4m3)
        xo = np.ascontiguousarray(
            xf.reshape(NBLK, 128, D)[r::N_CORES].reshape(512, D)).astype(
                ml_dtypes.bfloat16)
        in_maps.append({
            "xT": xT, "x_own": xo, "qkvT": qkvT_r, "o_wT": o_wT_full,
            "gT": gT, "uT": uT, "dT": dT,
        })
    return in_maps


def assemble_out(results):
    outf = np.empty((NTOK, D), np.float32)
    for r in range(N_CORES):
        outf.reshape(NBLK, 128, D)[r::N_CORES] = \
            results[r]["out"].reshape(4, 128, D)
    return outf.reshape(B, T, D)


# ===================== entry point =====================

_NC_CACHE = {}


def _get_nc():
    if "nc" not in _NC_CACHE:
        _NC_CACHE["nc"] = build_nc()
    return _NC_CACHE["nc"]


def kernel(x, ln1_w, ln2_w, qkv_w, o_w, gate_w, up_w, down_w):
    from concourse.bass_utils import run_bass_kernel_spmd

    nc = _get_nc()
    in_maps = make_in_maps(x, ln1_w, ln2_w, qkv_w, o_w, gate_w, up_w, down_w)
    res = run_bass_kernel_spmd(nc, in_maps, core_ids=list(range(N_CORES)))
    return assemble_out(res.results)



# revision 53
# speedup vs baseline: 1.6023x; 1.6023x over previous
"""Trainium2 Bass kernel for nn_MiniDecoderBlock (B=2, T=2048, D=1024, H=16, DI=2048).

Strategy: 8-way tensor-parallel attention (2 heads/core, both batches),
one chunked ReduceScatter of the o_proj partial sums distributing tokens,
then token-sharded FFN (512 tokens/core, full d_inner).

kernel(**inputs) takes the FULL unsharded inputs and returns the FULL
output; sharding/compile/run happen inside.
"""

"""MiniDecoderBlock Trainium kernel: TP-8 attention + RS + token-sharded FFN.

Layout conventions (device side, per core):
  - Activations feature-major: xT [D, tokens] so matmul contraction (partition
    dim) is the feature dim.
  - Scores computed transposed: scoresT [k_tokens(P), q_tokens(free)] so the
    PV matmul uses stationary V and lands yT feature-major for o_proj.
  - V stored token-major with an appended ones column (sumexp for free).
  - rmsnorm applied via a PE ones-broadcast of the rms row onto all partitions,
    multiplied into q/k/v at the mandatory PSUM->SBUF copy.
  - ReduceScatter distributes attention partial sums by token blocks; core r
    owns global 128-token blocks {8c + r}.
"""

import numpy as np

import concourse.bass as bass
import concourse.mybir as mybir
import concourse.tile as tile
from concourse import bacc
from concourse.masks import make_identity
from concourse.tile import TileContext

F32 = mybir.dt.float32
F32R = mybir.dt.float32r
F8 = mybir.dt.float8e4
BF16 = mybir.dt.bfloat16
DR = mybir.MatmulPerfMode.DoubleRow

N_CORES = 8
B, T, D = 2, 2048, 1024
H, HD = 16, 64
DI = 2048
HPC = H // N_CORES          # heads per core = 2
NTOK = B * T                # 4096
NCHUNK = NTOK // 512        # 8 x 512-token chunks
NBLK = NTOK // 128          # 32 x 128-token blocks
EPS = 1e-6
NEG = -1e30
SQKV = 64.0
NEG_LN_SQKV = -np.log(SQKV)


def r32(ap):
    return ap.bitcast(F32R)



def _pin_act_tables():
    import concourse.bacc as bacc_mod
    import concourse.hw_specs as hw_specs_mod
    import concourse.mybir as _mb
    orig = hw_specs_mod.get_activation_tables
    if getattr(bacc_mod.get_activation_tables, "_pinned", False):
        return
    AFT = _mb.ActivationFunctionType
    def patched(arch):
        t = orig(arch)
        out = {}
        for k, v in t.items():
            if k == "natural_log_exp_and_others":
                out[k] = set(v)
            else:
                out[k] = {f for f in v if f not in (AFT.Ln, AFT.Exp)}
        return out
    patched._pinned = True
    bacc_mod.get_activation_tables = patched

def build_nc(ffn_w_dtype=BF16, reps=1, no_collective=False):
    _pin_act_tables()
    nc = bacc.Bacc("TRN2", target_bir_lowering=False, debug=False,
                   num_devices=1 if no_collective else N_CORES)

    xT = nc.dram_tensor("xT", [NCHUNK, 128, 8 * 512], F8, kind="ExternalInput")
    x_own = nc.dram_tensor("x_own", [512, D], BF16, kind="ExternalInput")
    qkvT = nc.dram_tensor("qkvT", [D, 3 * HPC * HD], F8, kind="ExternalInput")
    o_wT = nc.dram_tensor("o_wT", [D, D], F8, kind="ExternalInput")
    gT = nc.dram_tensor("gT", [D, DI], F8, kind="ExternalInput")
    uT = nc.dram_tensor("uT", [D, DI], F8, kind="ExternalInput")
    dT = nc.dram_tensor("dT", [DI, D], F8, kind="ExternalInput")
    out = nc.dram_tensor("out", [512, D], F32, kind="ExternalOutput")

    with TileContext(nc) as tc:
        emit(nc, tc, xT, x_own, qkvT, o_wT, gT, uT, dT, out, reps=reps,
             no_collective=no_collective)
    nc.compile()
    return nc


def emit(nc, tc, xT, x_own, qkvT, o_wT, gT, uT, dT, out, reps=1, no_collective=False):
    EXP = mybir.ActivationFunctionType.Exp
    LN = mybir.ActivationFunctionType.Ln
    SQUARE = mybir.ActivationFunctionType.Square
    SILU = mybir.ActivationFunctionType.Silu
    MUL = mybir.AluOpType.mult
    ADD = mybir.AluOpType.add

    from contextlib import ExitStack
    ctx = ExitStack()
    consts = ctx.enter_context(tc.tile_pool(name="consts", bufs=1))
    dram = ctx.enter_context(tc.tile_pool(name="dram", bufs=1, space="DRAM"))
    psum = ctx.enter_context(tc.tile_pool(name="psum", bufs=2, space="PSUM"))
    sb = ctx.enter_context(tc.tile_pool(name="sb", bufs=2))

    # ---- constants ----
    ident_bf = consts.tile([128, 128], BF16, tag="ident_bf")
    make_identity(nc, ident_bf[:, :])
    eps_col = consts.tile([128, 1], F32, tag="eps_col")
    nc.vector.memset(eps_col[:, :], EPS)
    nl64_col = consts.tile([128, 1], F32, tag="nl64_col")
    nc.vector.memset(nl64_col[:, :], NEG_LN_SQKV)
    # PE-side causal masking: scores_diag = (-BIG*I).T @ ut01 + kT.T @ q
    # neg_ident = -BIG on the diagonal; ut01 = 1.0 strictly below diagonal (k>q)
    maddT = consts.tile([128, 128], BF16, tag="maddT")
    nc.gpsimd.memset(maddT[:, :], 0.0)
    nc.gpsimd.affine_select(
        out=maddT[:, :], in_=maddT[:, :],
        compare_op=mybir.AluOpType.is_ge, fill=NEG,
        base=0, pattern=[[-1, 128]], channel_multiplier=1,
    )

    # ---- persistent SBUF ----
    # Single DMAs with (p, blk, col)-ordered DRAM access patterns so each
    # load is one HWDGE instruction (~667ns queue cost each) instead of 8+.
    xo_sb = consts.tile([128, 4, D], BF16, tag="xo_sb")
    nc.sync.dma_start(out=xo_sb[:, :, :],
                      in_=x_own[:, :].rearrange("(blk p) c -> p blk c", p=128))
    qkvT_sb = consts.tile([128, 8, 384], F8, tag="qkvT_sb")
    nc.sync.dma_start(out=qkvT_sb[:, :, :],
                      in_=qkvT[:, :].rearrange("(kk p) c -> p kk c", p=128))
    # full o_w resident (same on every core), DoubleRow pair layout over the
    # y-dim: [:, pp, j, :] = rows (2pp+j)*128..(2pp+j+1)*128 of o_w.T
    # (loaded at i==0 so it stays off the startup critical path)
    o_wT_sb = consts.tile([128, 4, 2, D], F8, tag="o_wT_sb")

    kT_all = consts.tile([128, NTOK], BF16, tag="kT_all")
    # fp8 V, token-major, 80-wide slots (pair stride %16==0 for dual-fp8 LW): [:, h, gb, 0:64] = v for block gb,
    # col 64 = 1.0 (sumexp row). Adjacent blocks pair-slice for DoubleRow PV.
    v2 = consts.tile([128, HPC, NBLK, 80], F8, tag="v2")
    nc.vector.memset(v2[:, :, :, 64:65], 1.0)

    # FFN weights resident, fp8, DoubleRow pair layout: [:, p, j, :] holds
    # feature rows (2p+j)*128..(2p+j+1)*128 so lhsT slices [128,2,128] pair
    # two contraction blocks per matmul.
    g_sb = consts.tile([128, 4, 2, DI], gT.dtype, tag="g_sb")
    u_sb = consts.tile([128, 4, 2, DI], uT.dtype, tag="u_sb")
    d_sb = consts.tile([128, 8, 2, D], dT.dtype, tag="d_sb")

    # ---- DRAM bounce ----
    # y2 AllToAll: per chunk-pair c, core r receives every core's 2 heads of
    # y for ITS token block 8c+r. a2a[c, s] = [128 y-rows, 128 tokens].
    a2a_in = dram.tile([4, N_CORES, 128, 128], F8, tag="a2a_in")
    a2a_out = dram.tile([4, N_CORES, 128, 128], F8, tag="a2a_out")
    rms_own = dram.tile([4, 128], F32, tag="rms_own")
    # rms_all[r, c, :] = rms of global token block 8c+r (AllGather core-major)
    rms_all = dram.tile([8, 4, 128], F32, tag="rms_all")


    for _rep in range(reps):
        # ================= main loop over 512-token chunks =================
        def xt_load(i):
            # host pre-arranged to the exact SBUF layout: 4KB descriptors
            xt = sb.tile([128, 8, 512], F8, tag="xt", name=f"xt_{i}")
            nc.sync.dma_start(out=xt[:, :, :], in_=xT[i, :, :])
            return xt

        def rms_part(i):
            rms_row = sb.tile([1, 512], F32, tag="rms_row", name=f"rmsr_{i}")
            nc.gpsimd.dma_start(
                out=rms_row[0:1, :],
                in_=rms_all[(4 * i) % 8:(4 * i) % 8 + 4, i // 2, :])
            rms_b = sb.tile([128, 512], F32, tag="rms_b", name=f"rmsb_{i}")
            nc.gpsimd.partition_broadcast(rms_b[:, :], rms_row[0:1, :])
            return rms_b

        def stats(i):
            """Load xT chunk i + rms broadcast tile (emitted ~1.5 chunks
            ahead); rms values come from the AllGathered per-core stats."""
            return xt_load(i), rms_part(i)

        def qkv_steps(i, st):
            """Projection for chunk i as filler closures sprinkled into the
            previous chunk's attention g-loop (PE fills exp-wait gaps)."""
            csl = slice(i * 512, (i + 1) * 512)
            xt, rms_b = st
            state = {}
            steps = []

            def mk_proj(w, off, k2):
                def f():
                    if k2 == 0:
                        state[w] = psum.tile([128, 512], F32, tag="proj",
                                             bufs=2, name=f"pj_{w}_{i}")
                    nc.tensor.matmul(
                        state[w][:, :],
                        qkvT_sb[:, 2 * k2:2 * k2 + 2, off:off + 128],
                        xt[:, 2 * k2:2 * k2 + 2, :],
                        start=(k2 == 0), stop=(k2 == 3),
                        perf_mode=DR)
                return f

            def mk_epi(w):
                # per-projection epilogue so the proj psum ring can be 2 deep
                def f():
                    if w == "q":
                        q_sb = sb.tile([128, 512], BF16, tag="q_sb",
                                       name=f"q_{i}")
                        state["q_sb"] = q_sb
                        nc.vector.tensor_tensor(out=q_sb[:, :],
                                                in0=state["q"][:, :],
                                                in1=rms_b[:, :], op=MUL)
                    elif w == "k":
                        nc.vector.tensor_tensor(out=kT_all[:, csl],
                                                in0=state["k"][:, :],
                                                in1=rms_b[:, :], op=MUL)
                    else:
                        v_sb = sb.tile([128, 512], BF16, tag="v_sb",
                                       name=f"v_{i}")
                        state["v_sb"] = v_sb
                        nc.vector.tensor_tensor(out=v_sb[:, :],
                                                in0=state["v"][:, :],
                                                in1=rms_b[:, :], op=MUL)
                return f

            for w, off in (("q", 0), ("k", 128), ("v", 256)):
                for k2 in range(4):
                    steps.append(mk_proj(w, off, k2))
                steps.append(mk_epi(w))

            def mk_vt(h, j):
                def f():
                    gb = i * 4 + j
                    v_sb = state["v_sb"]
                    vt = psum.tile([128, 64], BF16, tag="proj", bufs=2)
                    nc.tensor.transpose(vt[:, :],
                                        v_sb[h * 64:(h + 1) * 64,
                                             j * 128:(j + 1) * 128],
                                        ident_bf[h * 64:(h + 1) * 64,
                                                  h * 64:(h + 1) * 64])
                    nc.vector.tensor_copy(v2[:, h, gb, 0:64], vt[:, :])
                return f

            for h in range(HPC):
                for j in range(4):
                    steps.append(mk_vt(h, j))
            return steps, state

        def attn_both(i, q_sb, y2_sb, fillers=()):
            fillers = list(fillers)
            b, li = divmod(i, 4)
            npair = 2 * li
            nstep = npair + 4
            per_g = max(1, -(-len(fillers) // nstep))
            yT = [psum.tile([128, 512], F32, tag="yT", bufs=2, name=f"yT_{i}_{h}")
                  for h in range(2)]

            def pop_some():
                for _ in range(per_g):
                    if fillers:
                        fillers.pop(0)()

            # ---- full k-block pairs: one exp per (head, pair), fp8 DR PV --
            for pi in range(npair):
                gb0 = b * 16 + 2 * pi
                scs = []
                for h in range(2):
                    scp = psum.tile([128, 2, 512], F32, tag="sc", bufs=2,
                                    name=f"scp{h}")
                    for jj in range(2):
                        nc.tensor.matmul(
                            scp[:, jj, :],
                            kT_all[h * 64:(h + 1) * 64,
                                   (gb0 + jj) * 128:(gb0 + jj + 1) * 128],
                            q_sb[h * 64:(h + 1) * 64, :],
                            start=True, stop=True)
                    scs.append(scp)
                for h in range(2):
                    pT = sb.tile([128, 2, 512], F8, tag="pT", bufs=3,
                                 name=f"pT{h}")
                    nc.scalar.activation(pT[:, :, :], scs[h][:, :, :], EXP)
                    nc.tensor.matmul(
                        yT[h][0:65, :],
                        v2[:, h, gb0:gb0 + 2, 0:65],
                        pT[:, :, :],
                        start=(pi == 0), stop=False, perf_mode=DR)
                pop_some()

            # ---- diagonal blocks: both heads share one psum pair-tile ----
            for d in range(4):
                g = li * 4 + d
                gb = b * 16 + g
                q_off = d * 128
                w = 512 - q_off
                sc = psum.tile([128, 2, 512], F32, tag="sc", bufs=2,
                               name="scd")
                for h in range(2):
                    nc.tensor.matmul(
                        sc[:, h, 0:w],
                        kT_all[h * 64:(h + 1) * 64, gb * 128:(gb + 1) * 128],
                        q_sb[h * 64:(h + 1) * 64, q_off:512],
                        start=True, stop=False)
                    nc.tensor.matmul(
                        sc[:, h, 0:128], maddT[:, :], ident_bf[:, :],
                        start=False, stop=True)
                pT = sb.tile([128, 2, 512], F8, tag="pT", bufs=3, name="pTd")
                nc.scalar.activation(pT[:, :, 0:w], sc[:, :, 0:w], EXP)
                for h in range(2):
                    nc.tensor.matmul(
                        yT[h][0:65, q_off:512],
                        v2[:, h, gb, 0:65],
                        pT[:, h, 0:w],
                        start=(li == 0 and d == 0), stop=(d == 3))
                pop_some()
            while fillers:
                fillers.pop(0)()
            for h in range(2):
                rec = sb.tile([1, 512], F32, tag="rec")
                nc.vector.reciprocal(rec[:, :], yT[h][64:65, :])
                rb = sb.tile([64, 512], F32, tag="rb", bufs=1)
                nc.gpsimd.partition_broadcast(rb[:, :], rec[0:1, :])
                # y2 = 16*y/sumexp (x16 for fp8 range; undone by the /1024)
                nc.vector.scalar_tensor_tensor(
                    out=y2_sb[:, h, :], in0=yT[h][0:64, :], scalar=16.0,
                    in1=rb[:, :], op0=MUL, op1=MUL)

        def y2_out(i, y2_sb):
            # chunk i holds blocks 4i..4i+3 -> destination cores 4(i%2)+jb of
            # the pair i//2; one DMA scatters all four.
            c, half = divmod(i, 2)
            for jb in range(4):
                nc.sync.dma_start(
                    out=a2a_in[c, 4 * half + jb].rearrange(
                        "(h p) t -> p h t", p=64),
                    in_=y2_sb[:, :, jb * 128:(jb + 1) * 128])

        def emit_a2a(c):
            if no_collective:
                nc.sync.dma_start(out=a2a_out[c], in_=a2a_in[c])
            else:
                nc.gpsimd.collective_compute(
                    "AllToAll", mybir.AluOpType.bypass,
                    ins=[a2a_in[c]],
                    outs=[a2a_out[c]],
                    replica_groups=[list(range(N_CORES))],
                )

        # ================= FFN on own 512 tokens =================
        def ffn_load(ha):
            """Fetch the AllToAll'd full-head y for own blocks ha*2, ha*2+1
            (a2a_out[c2] holds [8 src cores, 128 y-rows, 128 own tokens])."""
            loads = []
            for jj in range(2):
                c2 = ha * 2 + jj
                y_sb = sb.tile([128, 4, 2, 128], F8, tag="y_sb", bufs=2,
                               name=f"ysb{c2}")
                nc.sync.dma_start(
                    out=y_sb[:, :, :, :],
                    in_=a2a_out[c2].rearrange("(pp j) p t -> p pp j t", j=2))
                loads.append(y_sb)
            return loads

        def ffn_prep(ha, loads):
            x2t, xn2T, xn2s = [], [], []
            for jj in range(2):
                c2 = ha * 2 + jj
                y_sb = loads[jj]
                op2 = psum.tile([128, 2, 512], F32, tag="sc", bufs=2,
                                name=f"op2_{c2}")
                for n in range(2):
                    for pp in range(4):
                        nc.tensor.matmul(
                            op2[:, n, :],
                            y_sb[:, pp, :, :],
                            o_wT_sb[:, pp, :, n * 512:(n + 1) * 512],
                            start=(pp == 0), stop=(pp == 3),
                            perf_mode=DR)
                x2 = sb.tile([128, D], BF16, tag=f"x2_{jj}", bufs=2,
                             name=f"x2_{c2}")
                nc.vector.scalar_tensor_tensor(
                    out=x2[:, :], in0=op2[:, :, :], scalar=1.0 / 1024.0,
                    in1=xo_sb[:, c2, :], op0=MUL, op1=ADD)
                x2t.append(x2)
                scr = sb.tile([128, D], BF16, tag="scr", bufs=1, name=f"scr{c2}")
                ss2 = sb.tile([128, 1], F32, tag="ss2", name=f"ss2_{c2}")
                nc.scalar.activation(scr[:, :], x2[:, :], SQUARE,
                                     accum_out=ss2[:, :])
                t2 = sb.tile([128, 1], F32, tag="t2", name=f"t2_{c2}")
                nc.scalar.activation(t2[:, :], ss2[:, :], LN,
                                     bias=eps_col[:, :], scale=1.0 / D)
                r2 = sb.tile([128, 1], F32, tag="r2", name=f"r2_{c2}")
                nc.scalar.activation(r2[:, :], t2[:, :], EXP, scale=-0.5)
                xn2 = sb.tile([128, D], BF16, tag=f"xn2_{jj}", bufs=2,
                              name=f"xn2_{c2}")
                nc.vector.tensor_scalar_mul(xn2[:, :], x2[:, :], r2[:, :])
                xn2s.append(xn2)
            for p in range(4):
                xt2 = sb.tile([128, 2, 256], gT.dtype, tag=f"xn2T{p}",
                              bufs=2, name=f"xn2T{p}_{ha}")
                xn2T.append(xt2)

            def mk_tp(jj, kk):
                def f():
                    xp = psum.tile([128, 128], BF16, tag="proj", bufs=2)
                    nc.tensor.transpose(xp[:, :],
                                        xn2s[jj][:, kk * 128:(kk + 1) * 128],
                                        ident_bf[:, :])
                    nc.vector.tensor_copy(
                        xn2T[kk // 2][:, kk % 2, jj * 128:(jj + 1) * 128],
                        xp[:, :])
                return f

            tps = [mk_tp(jj, kk) for jj in range(2) for kk in range(8)]
            return x2t, xn2T, tps

        def ffn_gu_steps(ha, xn2T, gu_tag="sc"):
            """Gate/up m-loop as filler closures; gp+up share one psum bank
            (pipeline depth 2 within a 2-buf ring)."""
            h_all = sb.tile([128, 16, 256], F8, tag=f"h_all{ha}", bufs=1,
                            name=f"h_all_{ha}")

            def mk(m):
                def f():
                    gu = psum.tile([128, 2, 256], F32, tag=gu_tag, bufs=2,
                                   name=f"gu{ha}_{m}")
                    for p in range(4):
                        nc.tensor.matmul(gu[:, 0, :],
                                         g_sb[:, p, :, m * 128:(m + 1) * 128],
                                         xn2T[p][:, :, :],
                                         start=(p == 0), stop=(p == 3),
                                         perf_mode=DR)
                    for p in range(4):
                        nc.tensor.matmul(gu[:, 1, :],
                                         u_sb[:, p, :, m * 128:(m + 1) * 128],
                                         xn2T[p][:, :, :],
                                         start=(p == 0), stop=(p == 3),
                                         perf_mode=DR)
                    sg = sb.tile([128, 256], BF16, tag="sg")
                    nc.scalar.activation(sg[:, :], gu[:, 0, :], SILU,
                                         scale=1.0 / 64.0)
                    # h scaled x16 for fp8 (corrected with the down x64)
                    nc.vector.scalar_tensor_tensor(
                        out=h_all[:, m, :], in0=sg[:, :], scalar=0.5,
                        in1=gu[:, 1, :], op0=MUL, op1=MUL)
                return f

            return h_all, [mk(m) for m in range(16)]

        def ffn_down(ha, x2t, h_all):
            dp = [psum.tile([128, 512], F32,
                            tag="proj" if nn == 0 else "sc", bufs=2,
                            name=f"dp{jj}_{nn}")
                  for nn in range(2) for jj in range(2)]
            for m2 in range(8):
                for n in range(2):
                    for jj in range(2):
                        nc.tensor.matmul(dp[n * 2 + jj][:, :],
                                         h_all[:, 2 * m2:2 * m2 + 2,
                                               jj * 128:(jj + 1) * 128],
                                         d_sb[:, m2, :, n * 512:(n + 1) * 512],
                                         start=(m2 == 0), stop=(m2 == 7),
                                         perf_mode=DR)
            for n in range(2):
                for jj in range(2):
                    c2 = ha * 2 + jj
                    osb = sb.tile([128, 512], F32, tag="fout")
                    nc.vector.scalar_tensor_tensor(
                        out=osb[:, :], in0=dp[n * 2 + jj][:, :],
                        scalar=1.0 / 1024.0,
                        in1=x2t[jj][:, n * 512:(n + 1) * 512],
                        op0=MUL, op1=ADD)
                    nc.sync.dma_start(out=out[c2 * 128:(c2 + 1) * 128,
                                              n * 512:(n + 1) * 512],
                                      in_=osb[:, :])


        # ---- per-core rms of own 512 tokens, AllGathered to every core ----
        rmsq = sb.tile([128, 4], F32, tag="rmsq", bufs=1, name="rmsq")
        for blk in range(4):
            scr0 = sb.tile([128, D], BF16, tag="scr", bufs=1,
                           name=f"scr0_{blk}")
            ss0 = sb.tile([128, 1], F32, tag="ss2", name=f"ss0_{blk}")
            nc.scalar.activation(scr0[:, :], xo_sb[:, blk, :], SQUARE,
                                 accum_out=ss0[:, :])
            t0 = sb.tile([128, 1], F32, tag="t2", name=f"t0_{blk}")
            nc.scalar.activation(t0[:, :], ss0[:, :], LN,
                                 bias=eps_col[:, :], scale=1.0 / D)
            nc.scalar.activation(rmsq[:, blk:blk + 1], t0[:, :], EXP,
                                 scale=-0.5, bias=nl64_col[:, :])
            nc.gpsimd.dma_start(out=rms_own[blk:blk + 1, :],
                                in_=rmsq[:, blk:blk + 1])
        if no_collective:
            nc.gpsimd.dma_start(out=rms_all[0:1, :, :], in_=rms_own[:, :])
        else:
            nc.gpsimd.collective_compute(
                "AllGather", mybir.AluOpType.bypass,
                ins=[rms_own[:, :]], outs=[rms_all[:, :, :]],
                replica_groups=[list(range(N_CORES))],
            )

        st = stats(0)
        steps0, state0 = qkv_steps(0, st)
        for f in steps0:
            f()
        q_cur = state0["q_sb"]
        st_next = stats(1)
        state_next = None
        for i in range(NCHUNK):
            y2_sb = sb.tile([64, 2, 512], F8, tag="y2_sb", name=f"y2_{i}")
            if i + 1 < NCHUNK:
                fillers, state_next = qkv_steps(i + 1, st_next)
            else:
                fillers = []
            attn_both(i, q_cur, y2_sb, fillers)
            if i == 0:
                nc.sync.dma_start(
                    out=o_wT_sb[:, :, :, :],
                    in_=o_wT[:, :].rearrange("(pp j p) c -> p pp j c",
                                             p=128, j=2))
            if i == 1:
                nc.sync.dma_start(
                    out=g_sb[:, :, :, :],
                    in_=gT[:, :].rearrange("(pp j p) c -> p pp j c",
                                           p=128, j=2))
                nc.sync.dma_start(
                    out=u_sb[:, :, :, :],
                    in_=uT[:, :].rearrange("(pp j p) c -> p pp j c",
                                           p=128, j=2))
            if i == 2:
                nc.sync.dma_start(
                    out=d_sb[:, :, :, :],
                    in_=dT[:, :].rearrange("(m j p) c -> p m j c",
                                           p=128, j=2))
            if i + 2 < NCHUNK:
                st_next = stats(i + 2)
            y2_out(i, y2_sb)
            if i + 1 < NCHUNK:
                q_cur = state_next["q_sb"]
            if i % 2 == 1:
                emit_a2a(i // 2)
            if i == 5:
                ffn0 = ffn_prep(0, ffn_load(0))

        loads1 = ffn_load(1)        # waits on the final AllToAll (DMA only)
        x2t0, xn2T0, tps0 = ffn0
        for f in tps0:
            f()
        h_all0, gu0 = ffn_gu_steps(0, xn2T0)
        for f in gu0:
            f()
        x2t1, xn2T1, tps1 = ffn_prep(1, loads1)
        ffn_down(0, x2t0, h_all0)
        for f in tps1:
            f()
        h_all1, gu1 = ffn_gu_steps(1, xn2T1)
        for f in gu1:
            f()
        ffn_down(1, x2t1, h_all1)

    ctx.close()


# ===================== host-side sharding =====================

def make_in_maps(x, ln1_w, ln2_w, qkv_w, o_w, gate_w, up_w, down_w,
                 ffn_np_dtype=None):
    import ml_dtypes
    if ffn_np_dtype is None:
        ffn_np_dtype = ml_dtypes.bfloat16
    x = np.asarray(x, np.float32)
    xf = np.ascontiguousarray(x.reshape(NTOK, D))
    # chunk-major, SBUF-layout: xT[i, p, kk*512+t] = x[512i+t, kk*128+p]
    xT = np.ascontiguousarray(
        xf.reshape(NCHUNK, 512, 8, 128).transpose(0, 3, 2, 1).reshape(
            NCHUNK, 128, 8 * 512)).astype(ml_dtypes.float8_e4m3)

    qkv_eff = np.asarray(qkv_w, np.float32) * np.asarray(ln1_w, np.float32)[None, :]
    g_eff = np.asarray(gate_w, np.float32) * np.asarray(ln2_w, np.float32)[None, :]
    u_eff = np.asarray(up_w, np.float32) * np.asarray(ln2_w, np.float32)[None, :]
    o_w = np.asarray(o_w, np.float32)
    down_w = np.asarray(down_w, np.float32)

    gT = np.ascontiguousarray(g_eff.T * 64.0).astype(ml_dtypes.float8_e4m3)
    uT = np.ascontiguousarray(u_eff.T * 32.0).astype(ml_dtypes.float8_e4m3)
    dT = np.ascontiguousarray(down_w.T * 64.0).astype(ml_dtypes.float8_e4m3)

    scale = 1.0 / np.sqrt(HD)
    # full o_w.T (y-major), identical on every core
    o_wT_full = np.ascontiguousarray(o_w.T * 64.0).astype(
        ml_dtypes.float8_e4m3)
    in_maps = []
    for r in range(N_CORES):
        hsl = slice(r * HPC * HD, (r + 1) * HPC * HD)  # rows for this core's heads
        qr = qkv_eff[hsl, :] * scale          # [128, D] pre-scaled q
        kr = qkv_eff[D + r * 128:D + (r + 1) * 128, :]
        vr = qkv_eff[2 * D + r * 128:2 * D + (r + 1) * 128, :]
        qkvT_r = np.ascontiguousarray(
            np.concatenate([qr, kr, vr], axis=0).T * SQKV).astype(
                ml_dtypes.float8_e4m3)
        xo = np.ascontiguousarray(
            xf.reshape(NBLK, 128, D)[r::N_CORES].reshape(512, D)).astype(
                ml_dtypes.bfloat16)
        in_maps.append({
            "xT": xT, "x_own": xo, "qkvT": qkvT_r, "o_wT": o_wT_full,
            "gT": gT, "uT": uT, "dT": dT,
        })
    return in_maps


def assemble_out(results):
    outf = np.empty((NTOK, D), np.float32)
    for r in range(N_CORES):
        outf.reshape(NBLK, 128, D)[r::N_CORES] = \
            results[r]["out"].reshape(4, 128, D)
    return outf.reshape(B, T, D)


# ===================== entry point =====================

_NC_CACHE = {}


def _get_nc():
    if "nc" not in _NC_CACHE:
        _NC_CACHE["nc"] = build_nc()
    return _NC_CACHE["nc"]


def kernel(x, ln1_w, ln2_w, qkv_w, o_w, gate_w, up_w, down_w):
    from concourse.bass_utils import run_bass_kernel_spmd

    nc = _get_nc()
    in_maps = make_in_maps(x, ln1_w, ln2_w, qkv_w, o_w, gate_w, up_w, down_w)
    res = run_bass_kernel_spmd(nc, in_maps, core_ids=list(range(N_CORES)))
    return assemble_out(res.results)

